# revision 13
# baseline (speedup 1.0000x reference)
"""CNN-LSTM Trainium2 kernel (nn_CNNLSTM_59193239273595).

Data-parallel over 8 NeuronCores: batch 64 -> 8 sequences per core.
Per core:
  1. Embedding gather via dma_gather(transpose=True) on a bf16 copy of the
     table -> SBUF tiles laid out [E=128, L] (conv-ready, no on-chip
     transpose needed).
  2. Conv1d(E=128 -> F=64, K=5, VALID) as 5 PSUM-accumulated matmuls per
     512-wide chunk; maxpool(4) fused into PSUM evacuation (tensor_reduce)
     followed by relu+bias on ScalarE.
  3. LSTM input projections xg = conv_out @ w_ih.T + (b_ih + b_hh)
     precomputed for all T=1023 steps into SBUF (transposed gate layout).
  4. The 1023-step LSTM recurrence with the 8 local sequences split into
     two staggered groups of 4 so the per-step dependency chain of the two
     groups pipelines across engines.  Gates are computed in transposed
     [H=128, batch] layout; tanh(g) is computed as 2*sigmoid(2g)-1 with the
     doubling folded into the host-side weights, so one Sigmoid activation
     covers all four gates.
  5. FC head -> [C=2, 8] per core, assembled on host.

All matmuls run in bf16 (fp32 is 4x slower per PE row); PSUM accumulation
and the LSTM cell state stay fp32.
"""

import sys
from contextlib import ExitStack

if "/opt/trn_rl_repo" not in sys.path:
    sys.path.insert(0, "/opt/trn_rl_repo")

import numpy as np
import ml_dtypes

import concourse.bass as bass
import concourse.tile as tile
from concourse import bacc, mybir
from concourse.bass_utils import run_bass_kernel_spmd

BF16 = ml_dtypes.bfloat16

# Problem shapes (hardcoded per contract).
B, L = 64, 4096
VOCAB, E, F, K, P, H, C = 20000, 128, 64, 5, 4, 128, 2
NCORES = 8
S = B // NCORES          # sequences per core
LC = L - K + 1           # 4092
T = LC // P              # 1023
NCH = 8                  # conv chunks per sequence (7x512 + 508)
CHW = 512

F32 = mybir.dt.float32
BF = mybir.dt.bfloat16
I16 = mybir.dt.int16

AF = mybir.ActivationFunctionType
OP = mybir.AluOpType


def build_nc(T_steps: int = T):
    """Build the SPMD single-core program."""
    nc = bacc.Bacc("TRN2", target_bir_lowering=False, debug=False)

    # ---- DRAM I/O ----
    x_idx_d = nc.dram_tensor("x_idx", [S, 128, L // 16], I16, kind="ExternalInput")
    emb_d = nc.dram_tensor("emb_bf", [VOCAB, E], BF, kind="ExternalInput")
    convT_d = nc.dram_tensor("convT", [K, E, F], BF, kind="ExternalInput")
    convb_d = nc.dram_tensor("convb", [F, 1], F32, kind="ExternalInput")
    wihT_d = nc.dram_tensor("wihT", [4, F, H], BF, kind="ExternalInput")
    bihh_d = nc.dram_tensor("bihh", [4, H, 1], F32, kind="ExternalInput")
    whhT_d = nc.dram_tensor("whhT", [4, H, H], BF, kind="ExternalInput")
    ident_d = nc.dram_tensor("ident", [128, 128], BF, kind="ExternalInput")
    fcwT_d = nc.dram_tensor("fcwT", [H, C], BF, kind="ExternalInput")
    fcb_d = nc.dram_tensor("fcb", [C, 1], F32, kind="ExternalInput")
    out_d = nc.dram_tensor("out", [C, S], F32, kind="ExternalOutput")

    with tile.TileContext(nc) as tc, ExitStack() as st:
        wp = st.enter_context(tc.tile_pool(name="weights", bufs=1))
        idxp = st.enter_context(tc.tile_pool(name="idx", bufs=3))
        embp = st.enter_context(tc.tile_pool(name="emb", bufs=2))
        cop = st.enter_context(tc.tile_pool(name="convout", bufs=1))
        xgp = st.enter_context(tc.tile_pool(name="xg", bufs=1))
        stp = st.enter_context(tc.tile_pool(name="state", bufs=1))
        outp = st.enter_context(tc.tile_pool(name="outp", bufs=1))

        # ---- load weights to SBUF ----
        convT_sb = wp.tile([E, K * F], BF, tag="convT")
        for k in range(K):
            nc.sync.dma_start(convT_sb[:, k * F:(k + 1) * F], convT_d.ap()[k])
        convb_sb = wp.tile([F, 1], F32, tag="convb")
        nc.sync.dma_start(convb_sb[:], convb_d.ap()[:])
        wihT_sb = wp.tile([F, 4 * H], BF, tag="wihT")
        for g in range(4):
            nc.sync.dma_start(wihT_sb[:, g * H:(g + 1) * H], wihT_d.ap()[g])
        bihh_sb = wp.tile([H, 4], F32, tag="bihh")
        for g in range(4):
            nc.sync.dma_start(bihh_sb[:, g:g + 1], bihh_d.ap()[g])
        whhT_sb = wp.tile([H, 4 * H], BF, tag="whhT")
        for g in range(4):
            nc.sync.dma_start(whhT_sb[:, g * H:(g + 1) * H], whhT_d.ap()[g])
        ident_sb = wp.tile([128, 128], BF, tag="ident")
        nc.sync.dma_start(ident_sb[:], ident_d.ap()[:])
        fcwT_sb = wp.tile([H, C], BF, tag="fcwT")
        nc.sync.dma_start(fcwT_sb[:], fcwT_d.ap()[:])
        fcb_sb = wp.tile([C, 1], F32, tag="fcb")
        nc.sync.dma_start(fcb_sb[:], fcb_d.ap()[:])

        # conv_out for all S sequences: [F, S*1024] (1023 cols used per seq)
        conv_sb = cop.tile([F, S * 1024], BF, tag="convout")
        # xg for all steps: [128, T*32] bf16;
        # col layout: t*32 + group*16 + gate*4 + lane
        xg_sb = xgp.tile([128, T_steps * 32], BF, tag="xg")
        xg3 = xg_sb[:].rearrange("p (t c) -> p t c", c=32)

        # ---- phase 1+2+3 per sequence ----
        with (
            tc.tile_pool(name="cvps", bufs=3, space="PSUM") as cvps,
            tc.tile_pool(name="xgps", bufs=2, space="PSUM") as xgps,
            tc.tile_pool(name="mp", bufs=4) as mpp,
        ):
            for s in range(S):
                idx_t = idxp.tile([128, L // 16], I16, tag="idx")
                nc.sync.dma_start(idx_t[:], x_idx_d.ap()[s])
                embT = embp.tile([128, 1, L], BF, tag="embT")
                nc.gpsimd.dma_gather(
                    embT[:], emb_d.ap()[:], idx_t[:], L, L, E, transpose=True,
                    single_packet=False,
                )
                # conv + maxpool + relu
                for cchunk in range(NCH):
                    c0 = cchunk * CHW
                    W = min(CHW, LC - c0)          # 512 or 508
                    ps = cvps.tile([F, CHW], F32, tag="cvps")
                    for k in range(K):
                        nc.tensor.matmul(
                            ps[:, :W],
                            convT_sb[:, k * F:(k + 1) * F],
                            embT[:, 0, c0 + k: c0 + k + W],
                            start=(k == 0),
                            stop=(k == K - 1),
                        )
                    Wp = W // P                    # 128 or 127
                    mp = mpp.tile([F, CHW // P], F32, tag="mp")
                    nc.vector.tensor_reduce(
                        mp[:, :Wp],
                        ps[:, :Wp * P].rearrange("p (a b) -> p a b", b=P),
                        axis=mybir.AxisListType.X,
                        op=OP.max,
                    )
                    nc.scalar.activation(
                        conv_sb[:, s * 1024 + cchunk * 128:
                                s * 1024 + cchunk * 128 + Wp],
                        mp[:, :Wp],
                        AF.Relu,
                        bias=convb_sb[:, 0:1],
                    )
                # xg = conv_out @ w_ih.T + bias   (transposed layout)
                grp, lane = divmod(s, 4)
                for g in range(4):
                    for tch in range(2):
                        t0 = tch * CHW
                        Wt = min(CHW, T_steps - t0)
                        if Wt <= 0:
                            continue
                        psx = xgps.tile([H, CHW], F32, tag="xgps")
                        nc.tensor.matmul(
                            psx[:, :Wt],
                            wihT_sb[:, g * H:(g + 1) * H],
                            conv_sb[:F, s * 1024 + t0: s * 1024 + t0 + Wt],
                            start=True,
                            stop=True,
                        )
                        nc.vector.tensor_scalar(
                            xg3[:, t0:t0 + Wt, grp * 16 + g * 4 + lane],
                            psx[:, :Wt],
                            bihh_sb[:, g:g + 1],
                            None,
                            OP.add,
                        )

        # ---- phase 4: LSTM ----
        with (
            tc.tile_pool(name="lstmps", bufs=4, space="PSUM") as lps,
            tc.tile_pool(name="sigs", bufs=4) as sgp,
            tc.tile_pool(name="ltmp", bufs=4) as ltp,
        ):
            c_states = [
                stp.tile([H, 4], F32, tag="c_state_a", name="c_state_a"),
                stp.tile([H, 4], F32, tag="c_state_b", name="c_state_b"),
            ]
            h_states = [
                stp.tile([H, 4], BF, tag="h_state_a", name="h_state_a"),
                stp.tile([H, 4], BF, tag="h_state_b", name="h_state_b"),
            ]
            for grp in range(2):
                nc.vector.memset(c_states[grp][:], 0.0)
                nc.vector.memset(h_states[grp][:], 0.0)

            # Software-pipelined half-steps: each group's step is split into
            # head (matmuls + sigmoid + c-update) and tail (tanh + h-update);
            # the other group's tail is emitted between heads so each
            # engine's in-order queue alternates between the two independent
            # chains.
            def head(grp, t):
                ps = lps.tile([128, 16], F32, tag="lstmps")
                nc.tensor.matmul(
                    ps[:],
                    ident_sb[:],
                    xg3[:, t, grp * 16:(grp + 1) * 16],
                    start=True,
                    stop=False,
                )
                for g in range(4):
                    nc.tensor.matmul(
                        ps[:, g * 4:(g + 1) * 4],
                        whhT_sb[:, g * H:(g + 1) * H],
                        h_states[grp][:],
                        start=False,
                        stop=(g == 3),
                    )
                sg = sgp.tile([128, 16], F32, tag="sigs")
                nc.scalar.activation(sg[:], ps[:], AF.Sigmoid)
                # gates: cols 0:4=i, 4:8=f, 8:12=o, 12:16=sigma(2g)
                m = ltp.tile([H, 4], F32, tag="m")
                nc.vector.scalar_tensor_tensor(
                    m[:], sg[:, 12:16], 0.5, sg[:, 0:4], OP.subtract, OP.mult,
                )
                fcv = ltp.tile([H, 4], F32, tag="fcv")
                nc.vector.tensor_mul(fcv[:], sg[:, 4:8], c_states[grp][:])
                nc.vector.scalar_tensor_tensor(
                    c_states[grp][:], m[:], 2.0, fcv[:], OP.mult, OP.add,
                )
                return sg

            def tail(grp, sg):
                tch_t = ltp.tile([H, 4], F32, tag="tc")
                nc.scalar.activation(tch_t[:], c_states[grp][:], AF.Tanh)
                nc.vector.tensor_mul(h_states[grp][:], sg[:, 8:12], tch_t[:])

            pending = {}
            for t in range(T_steps):
                for grp in range(2):
                    sg = head(grp, t)
                    other = 1 - grp
                    if other in pending:
                        tail(other, pending.pop(other))
                    pending[grp] = sg
            for grp, sg in sorted(pending.items()):
                tail(grp, sg)

            # ---- phase 5: FC ----
            psf = lps.tile([C, 16], F32, tag="lstmps")
            for grp in range(2):
                nc.tensor.matmul(
                    psf[:, grp * 4:(grp + 1) * 4],
                    fcwT_sb[:],
                    h_states[grp][:],
                    start=(grp == 0),
                    stop=(grp == 1),
                )
            out_sb = outp.tile([C, S], F32, tag="out")
            nc.scalar.activation(
                out_sb[:], psf[:, :8], AF.Identity, bias=fcb_sb[:, 0:1]
            )
            nc.sync.dma_start(out_d.ap()[:], out_sb[:])

    nc.compile()
    return nc


def prep_inputs(x, emb, conv_w, conv_b, w_ih, w_hh, b_ih, b_hh, fc_w, fc_b):
    """Host-side prep: per-core in_maps for run_bass_kernel_spmd."""
    x = np.asarray(x)
    emb = np.asarray(emb, np.float32)
    conv_w = np.asarray(conv_w, np.float32)
    conv_b = np.asarray(conv_b, np.float32)
    w_ih = np.asarray(w_ih, np.float32)
    w_hh = np.asarray(w_hh, np.float32)
    b_ih = np.asarray(b_ih, np.float32)
    b_hh = np.asarray(b_hh, np.float32)
    fc_w = np.asarray(fc_w, np.float32)
    fc_b = np.asarray(fc_b, np.float32)

    # gate order [i, f, o, g]; the "g" gate row-block is scaled by 2 for the
    # tanh(x) = 2*sigmoid(2x) - 1 trick.
    slices = [slice(0, H), slice(H, 2 * H), slice(3 * H, 4 * H), slice(2 * H, 3 * H)]
    scales = [1.0, 1.0, 1.0, 2.0]

    whhT = np.stack(
        [(w_hh[sl] * sc).T.astype(BF16) for sl, sc in zip(slices, scales)]
    )  # [4, H, H]
    wihT = np.stack(
        [(w_ih[sl] * sc).T.astype(BF16) for sl, sc in zip(slices, scales)]
    )  # [4, F, H]
    bihh = np.stack(
        [((b_ih + b_hh)[sl] * sc).astype(np.float32)[:, None]
         for sl, sc in zip(slices, scales)]
    )  # [4, H, 1]

    convT = np.stack(
        [conv_w[:, :, k].T.astype(BF16) for k in range(K)]
    )  # [K, E, F]

    shared = {
        "emb_bf": emb.astype(BF16),
        "convT": convT,
        "convb": conv_b.astype(np.float32)[:, None],
        "wihT": wihT,
        "bihh": bihh,
        "whhT": whhT,
        "ident": np.eye(128, dtype=BF16),
        "fcwT": fc_w.T.astype(BF16),
        "fcb": fc_b.astype(np.float32)[:, None],
    }

    in_maps = []
    for c in range(NCORES):
        xc = np.asarray(x[c * S:(c + 1) * S], np.int64)
        # wrapped layout: idx token i lives at [i % 16, i // 16], replicated
        # over the 8 groups of 16 partitions.
        xr = xc.reshape(S, L // 16, 16).transpose(0, 2, 1)   # [S, 16, L/16]
        x_idx = np.tile(xr, (1, 8, 1)).astype(np.int16)       # [S, 128, L/16]
        in_maps.append({"x_idx": x_idx, **shared})
    return in_maps


_NC_CACHE = {}


def _get_nc():
    if "nc" not in _NC_CACHE:
        _NC_CACHE["nc"] = build_nc()
    return _NC_CACHE["nc"]


def _assemble(results):
    out = np.zeros((B, C), np.float32)
    for c in range(NCORES):
        out[c * S:(c + 1) * S] = results[c]["out"].T
    return out


def run(inputs, trace=False):
    nc = _get_nc()
    in_maps = prep_inputs(**inputs)
    res = run_bass_kernel_spmd(nc, in_maps, list(range(NCORES)), trace=trace)
    return _assemble(res.results), res


def kernel(**inputs) -> np.ndarray:
    out, _ = run(inputs)
    return out
